# revision 3
# baseline (speedup 1.0000x reference)
"""ALNN layer on 8 TRN2 NeuronCores.

Math (per reference):
  ref_r = linspace(0, 48, 64);  a_r = relu(alpha_r)
  e[b,r,l,d]  = exp(-a_r * |T[b,l,d] - ref_r|)
  intensity   = relu(X * e) = relu(X) * e            (e > 0 always)
  p[b,r,l,d]  = w0*X + w1*relu(X)*e + w2*M + w3*DT + w4*P + 5*b_t[r,l,d]
  h           = relu(p)
  out[b,r,d]  = relu( sum_l w_v[r,l,d]*h + 128*b_v[r,d] )

Sharding: R=64 split 8 ways (8 r per core); inputs replicated.
Layout: partition = L (128), free = (b, d) = (32, 48); loop over the 8 local r.
Engines: ACT does Abs/Exp/Relu (per-r scalars as [P,1] bias/scale);
DVE + GPSIMD split the 12 tensor-tensor ops; PE does the sum over L via
one-hot-column matmuls accumulating all 8 r into one PSUM tile, with the
128*b_v bias folded in as an identity-matmul that opens the accumulation.
"""
import sys

import numpy as np

if "/opt/trn_rl_repo" not in sys.path:
    sys.path.insert(0, "/opt/trn_rl_repo")

import ml_dtypes

from concourse import bacc, mybir
import concourse.tile as tile
from concourse.bass_utils import run_bass_kernel_spmd

BF16 = ml_dtypes.bfloat16
B, L, D = 32, 128, 48
R = 64
RL = R // 8  # r per core
INIT_TIME, MAX_TS = 0.0, 48.0

_CACHE = {}


def _build():
    nc = bacc.Bacc("TRN2", target_bir_lowering=False, debug=False, num_devices=8)
    f32, bf16 = mybir.dt.float32, mybir.dt.bfloat16
    AF = mybir.ActivationFunctionType

    # DRAM parameters (per-core shards / replicas)
    dTt = nc.dram_tensor("Tt", [L, B, D], f32, kind="ExternalInput").ap()
    dX = nc.dram_tensor("Xb", [L, B, D], bf16, kind="ExternalInput").ap()
    dM = nc.dram_tensor("Mb", [L, B, D], bf16, kind="ExternalInput").ap()
    dDT = nc.dram_tensor("DTb", [L, B, D], bf16, kind="ExternalInput").ap()
    dP = nc.dram_tensor("Pb", [L, B, D], bf16, kind="ExternalInput").ap()
    dWK = nc.dram_tensor("WK", [L, RL, 6, D], bf16, kind="ExternalInput").ap()
    dWV = nc.dram_tensor("WV", [L, RL, D], bf16, kind="ExternalInput").ap()
    dRN = nc.dram_tensor("RN", [L, RL], f32, kind="ExternalInput").ap()
    dAN = nc.dram_tensor("AN", [L, RL], f32, kind="ExternalInput").ap()
    dBV = nc.dram_tensor("BVl", [D, RL], bf16, kind="ExternalInput").ap()
    dID = nc.dram_tensor("ID48", [D, D], bf16, kind="ExternalInput").ap()
    dOUT = nc.dram_tensor("out", [B, RL, D], f32, kind="ExternalOutput").ap()

    NCH = 4  # psum chunks of 8 b each (8*48 = 384 floats < 512/bank)
    BC = B // NCH

    with tile.TileContext(nc) as tc:
        with (
            tc.tile_pool(name="const", bufs=1) as cpool,
            tc.tile_pool(name="work", bufs=2) as wpool,
            tc.tile_pool(name="psum", bufs=1, space="PSUM") as ppool,
            tc.tile_pool(name="outp", bufs=1) as opool,
        ):
            # ---- load persistent inputs
            tT = cpool.tile([L, B, D], f32, tag="T")
            nc.sync.dma_start(tT[:], dTt)
            tX = cpool.tile([L, B, D], bf16, tag="X")
            nc.sync.dma_start(tX[:], dX)
            tM = cpool.tile([L, B, D], bf16, tag="M")
            nc.sync.dma_start(tM[:], dM)
            tDT = cpool.tile([L, B, D], bf16, tag="DT")
            nc.sync.dma_start(tDT[:], dDT)
            tP = cpool.tile([L, B, D], bf16, tag="P")
            nc.sync.dma_start(tP[:], dP)
            tWK = cpool.tile([L, RL, 6, D], bf16, tag="WK")
            nc.sync.dma_start(tWK[:], dWK)
            tWV = cpool.tile([L, RL, D], bf16, tag="WV")
            nc.sync.dma_start(tWV[:], dWV)
            tRN = cpool.tile([L, RL], f32, tag="RN")
            nc.sync.dma_start(tRN[:], dRN)
            tAN = cpool.tile([L, RL], f32, tag="AN")
            nc.sync.dma_start(tAN[:], dAN)
            tBV = cpool.tile([D, RL], bf16, tag="BV")
            nc.sync.dma_start(tBV[:], dBV)
            tID = cpool.tile([D, D], bf16, tag="ID")
            nc.sync.dma_start(tID[:], dID)

            # xp = relu(X)
            tXP = cpool.tile([L, B, D], bf16, tag="XP")
            nc.scalar.activation(tXP[:], tX[:], AF.Relu)

            # one-hot lhsT matrices: oh[l, j, m] = (m == j)
            tOH = cpool.tile([L, RL, RL], bf16, tag="OH")
            nc.vector.memset(tOH[:], 0.0)
            for j in range(RL):
                nc.vector.memset(tOH[:, j, j : j + 1], 1.0)

            # psum accumulators, one bank-sized chunk of (b, d) each
            psc = [
                ppool.tile([RL, BC, D], mybir.dt.float32, tag=f"ps{c}", name=f"ps{c}")
                for c in range(NCH)
            ]
            # open each accumulation group with the bias term:
            # ps[r, b, d] = 128*b_v[r, d] via lhsT=BVl[d', r], rhs=Id[d', (b d)]
            for c in range(NCH):
                nc.tensor.matmul(
                    psc[c][:],
                    tBV[:],
                    tID[:, None, :].to_broadcast((D, BC, D)),
                    start=True,
                    stop=False,
                )

            shp = (L, B, D)
            for j in range(RL):
                # DMA-expand this r's weights to flat [L, B, D] tiles so every
                # hot DVE op is a contiguous bf16 tensor_tensor (2x mode);
                # broadcast-AP operands on DVE run ~4x slower.
                wf = {}
                for k in (0, 1, 2, 4, 5):
                    wf[k] = wpool.tile([L, B, D], bf16, tag=f"wf{k}", name=f"wf{k}_{j}")
                    nc.sync.dma_start(
                        wf[k][:], tWK[:, j : j + 1, k].to_broadcast(shp)
                    )
                wvf = wpool.tile([L, B, D], bf16, tag="wvf", name=f"wvf_{j}")
                nc.sync.dma_start(
                    wvf[:], tWV[:, j : j + 1, :].to_broadcast(shp)
                )

                dist = wpool.tile([L, B, D], f32, tag="dist")
                nc.scalar.activation(dist[:], tT[:], AF.Abs, bias=tRN[:, j : j + 1])
                ebf = wpool.tile([L, B, D], bf16, tag="ebf")
                nc.scalar.activation(ebf[:], dist[:], AF.Exp, scale=tAN[:, j : j + 1])

                q = wpool.tile([L, B, D], bf16, tag="q")
                nc.vector.tensor_mul(q[:], tXP[:], wf[1][:])
                t = wpool.tile([L, B, D], bf16, tag="t")
                nc.vector.tensor_mul(t[:], q[:], ebf[:])

                a0 = wpool.tile([L, B, D], bf16, tag="a0")
                nc.vector.tensor_mul(a0[:], tX[:], wf[0][:])
                a2 = wpool.tile([L, B, D], bf16, tag="a2")
                nc.gpsimd.tensor_mul(a2[:], tM[:], wf[2][:])
                a3 = wpool.tile([L, B, D], bf16, tag="a3")
                nc.gpsimd.tensor_mul(
                    a3[:], tDT[:], tWK[:, j : j + 1, 3].to_broadcast(shp)
                )
                a4 = wpool.tile([L, B, D], bf16, tag="a4")
                nc.vector.tensor_mul(a4[:], tP[:], wf[4][:])

                s1 = wpool.tile([L, B, D], bf16, tag="s1")
                nc.vector.tensor_add(s1[:], a0[:], a4[:])
                s2 = wpool.tile([L, B, D], bf16, tag="s2")
                nc.vector.tensor_add(s2[:], a2[:], a3[:])
                s3 = wpool.tile([L, B, D], bf16, tag="s3")
                nc.vector.tensor_add(s3[:], s1[:], s2[:])
                s4 = wpool.tile([L, B, D], bf16, tag="s4")
                nc.vector.tensor_add(s4[:], s3[:], wf[5][:])
                p = wpool.tile([L, B, D], bf16, tag="p")
                nc.vector.tensor_add(p[:], s4[:], t[:])

                h = wpool.tile([L, B, D], bf16, tag="h")
                nc.scalar.activation(h[:], p[:], AF.Relu)
                wh = wpool.tile([L, B, D], bf16, tag="wh")
                nc.vector.tensor_mul(wh[:], h[:], wvf[:])

                for c in range(NCH):
                    nc.tensor.matmul(
                        psc[c][:],
                        tOH[:, j, :],
                        wh[:, c * BC : (c + 1) * BC, :],
                        start=False,
                        stop=(j == RL - 1),
                    )

            # epilogue: relu(psum) -> sbuf f32, DMA out
            outf = opool.tile([RL, B, D], mybir.dt.float32, tag="outf")
            for c in range(NCH):
                nc.scalar.activation(
                    outf[:, c * BC : (c + 1) * BC, :], psc[c][:], AF.Relu
                )
            nc.sync.dma_start(dOUT.transpose([1, 0, 2]), outf[:])

    nc.compile()
    return nc


def _prep(X, T, M, DT, P, alpha, w_t, b_t, w_v, b_v):
    """Host-side shard prep: returns in_maps for the 8 cores."""
    refs = np.linspace(INIT_TIME, MAX_TS, R, dtype=np.float32)
    arelu = np.maximum(alpha.reshape(R).astype(np.float32), 0.0)

    Tt = np.ascontiguousarray(T.transpose(1, 0, 2)).astype(np.float32)
    Xb = np.ascontiguousarray(X.transpose(1, 0, 2)).astype(BF16)
    Mb = np.ascontiguousarray(M.transpose(1, 0, 2)).astype(BF16)
    DTb = np.ascontiguousarray(DT.transpose(1, 0, 2)).astype(BF16)
    Pb = np.ascontiguousarray(P.transpose(1, 0, 2)).astype(BF16)
    id48 = np.eye(D, dtype=np.float32).astype(BF16)

    # WK[l, j, k, d]: k<5 -> w_t[r, l, d, k]; k=5 -> 5*b_t[r, l, d, 0]
    wk_full = np.concatenate(
        [w_t, 5.0 * b_t], axis=3
    )  # [R, L, D, 6]
    in_maps = []
    for i in range(8):
        r0 = i * RL
        wk = np.ascontiguousarray(
            wk_full[r0 : r0 + RL].transpose(1, 0, 3, 2)
        ).astype(BF16)  # [L, RL, 6, D]
        wv = np.ascontiguousarray(
            w_v[r0 : r0 + RL].transpose(1, 0, 2)
        ).astype(BF16)  # [L, RL, D]
        rn = np.broadcast_to(-refs[r0 : r0 + RL], (L, RL)).astype(np.float32)
        an = np.broadcast_to(-arelu[r0 : r0 + RL], (L, RL)).astype(np.float32)
        bvl = np.ascontiguousarray(
            (128.0 * b_v[r0 : r0 + RL, 0, :]).T
        ).astype(BF16)  # [D, RL]
        in_maps.append(
            {
                "Tt": Tt,
                "Xb": Xb,
                "Mb": Mb,
                "DTb": DTb,
                "Pb": Pb,
                "WK": np.ascontiguousarray(wk),
                "WV": np.ascontiguousarray(wv),
                "RN": np.ascontiguousarray(rn),
                "AN": np.ascontiguousarray(an),
                "BVl": bvl,
                "ID48": id48,
            }
        )
    return in_maps


def run(trace=False, **inputs):
    if "nc" not in _CACHE:
        _CACHE["nc"] = _build()
    nc = _CACHE["nc"]
    in_maps = _prep(**inputs)
    res = run_bass_kernel_spmd(nc, in_maps, core_ids=list(range(8)), trace=trace)
    out = np.empty((B, R, D), dtype=np.float32)
    for i in range(8):
        out[:, i * RL : (i + 1) * RL, :] = res.results[i]["out"]
    return out, res


def kernel(**inputs) -> np.ndarray:
    out, _ = run(trace=False, **inputs)
    return out


# revision 4
# speedup vs baseline: 1.2935x; 1.2935x over previous
"""ALNN layer on 8 TRN2 NeuronCores.

Math (per reference):
  ref_r = linspace(0, 48, 64);  a_r = relu(alpha_r)
  e[b,r,l,d]  = exp(-a_r * |T[b,l,d] - ref_r|)
  intensity   = relu(X * e) = relu(X) * e            (e > 0 always)
  p[b,r,l,d]  = w0*X + w1*relu(X)*e + w2*M + w3*DT + w4*P + 5*b_t[r,l,d]
  h           = relu(p)
  out[b,r,d]  = relu( sum_l w_v[r,l,d]*h + 128*b_v[r,d] )

Sharding: R=64 split 8 ways (8 r per core); inputs replicated.
Layout: partition = L (128), free = (b, d) = (32, 48); loop over the 8 local r.
Engines: ACT does Abs/Exp/Relu (per-r scalars as [P,1] bias/scale);
DVE + GPSIMD split the 12 tensor-tensor ops; PE does the sum over L via
one-hot-column matmuls accumulating all 8 r into one PSUM tile, with the
128*b_v bias folded in as an identity-matmul that opens the accumulation.
"""
import sys

import numpy as np

if "/opt/trn_rl_repo" not in sys.path:
    sys.path.insert(0, "/opt/trn_rl_repo")

import ml_dtypes

from concourse import bacc, mybir
import concourse.tile as tile
from concourse.bass_utils import run_bass_kernel_spmd

BF16 = ml_dtypes.bfloat16
B, L, D = 32, 128, 48
R = 64
RL = R // 8  # r per core
INIT_TIME, MAX_TS = 0.0, 48.0

_CACHE = {}


def _build():
    nc = bacc.Bacc("TRN2", target_bir_lowering=False, debug=False, num_devices=8)
    f32, bf16 = mybir.dt.float32, mybir.dt.bfloat16
    AF = mybir.ActivationFunctionType

    # DRAM parameters (per-core shards / replicas)
    dTt = nc.dram_tensor("Tt", [L, B, D], f32, kind="ExternalInput").ap()
    dX = nc.dram_tensor("Xb", [L, B, D], bf16, kind="ExternalInput").ap()
    dM = nc.dram_tensor("Mb", [L, B, D], bf16, kind="ExternalInput").ap()
    dDT = nc.dram_tensor("DTb", [L, B, D], bf16, kind="ExternalInput").ap()
    dP = nc.dram_tensor("Pb", [L, B, D], bf16, kind="ExternalInput").ap()
    dWX = nc.dram_tensor("WX", [RL, 7, L, 16, D], bf16, kind="ExternalInput").ap()
    dRN = nc.dram_tensor("RN", [L, RL], f32, kind="ExternalInput").ap()
    dAN = nc.dram_tensor("AN", [L, RL], f32, kind="ExternalInput").ap()
    dBV = nc.dram_tensor("BVl", [D, RL], bf16, kind="ExternalInput").ap()
    dID = nc.dram_tensor("ID48", [D, D], bf16, kind="ExternalInput").ap()
    dOUT = nc.dram_tensor("out", [B, RL, D], f32, kind="ExternalOutput").ap()

    NCH = 4  # psum chunks of 8 b each (8*48 = 384 floats < 512/bank)
    BC = B // NCH

    with tile.TileContext(nc) as tc:
        with (
            tc.tile_pool(name="const", bufs=1) as cpool,
            tc.tile_pool(name="work", bufs=2) as wpool,
            tc.tile_pool(name="psum", bufs=1, space="PSUM") as ppool,
            tc.tile_pool(name="outp", bufs=1) as opool,
        ):
            # ---- load persistent inputs
            tT = cpool.tile([L, B, D], f32, tag="T")
            nc.sync.dma_start(tT[:], dTt)
            tX = cpool.tile([L, B, D], bf16, tag="X")
            nc.sync.dma_start(tX[:], dX)
            tM = cpool.tile([L, B, D], bf16, tag="M")
            nc.sync.dma_start(tM[:], dM)
            tDT = cpool.tile([L, B, D], bf16, tag="DT")
            nc.sync.dma_start(tDT[:], dDT)
            tP = cpool.tile([L, B, D], bf16, tag="P")
            nc.sync.dma_start(tP[:], dP)
            tRN = cpool.tile([L, RL], f32, tag="RN")
            nc.sync.dma_start(tRN[:], dRN)
            tAN = cpool.tile([L, RL], f32, tag="AN")
            nc.sync.dma_start(tAN[:], dAN)
            tBV = cpool.tile([D, RL], bf16, tag="BV")
            nc.sync.dma_start(tBV[:], dBV)
            tID = cpool.tile([D, D], bf16, tag="ID")
            nc.sync.dma_start(tID[:], dID)

            # xp = relu(X)
            tXP = cpool.tile([L, B, D], bf16, tag="XP")
            nc.scalar.activation(tXP[:], tX[:], AF.Relu)

            # one-hot lhsT matrices: oh[l, j, m] = (m == j)
            tOH = cpool.tile([L, RL, RL], bf16, tag="OH")
            nc.vector.memset(tOH[:], 0.0)
            for j in range(RL):
                nc.vector.memset(tOH[:, j, j : j + 1], 1.0)

            # psum accumulators, one bank-sized chunk of (b, d) each
            psc = [
                ppool.tile([RL, BC, D], mybir.dt.float32, tag=f"ps{c}", name=f"ps{c}")
                for c in range(NCH)
            ]
            # open each accumulation group with the bias term:
            # ps[r, b, d] = 128*b_v[r, d] via lhsT=BVl[d', r], rhs=Id[d', (b d)]
            for c in range(NCH):
                nc.tensor.matmul(
                    psc[c][:],
                    tBV[:],
                    tID[:, None, :].to_broadcast((D, BC, D)),
                    start=True,
                    stop=False,
                )

            H = 16  # weights expanded over half of B; ops run per b-half
            for j in range(RL):
                # per-r weights, host-pre-expanded to flat [L, 16, D] bf16 tiles
                wf = {}
                for k in range(7):
                    wf[k] = wpool.tile([L, H, D], bf16, tag=f"wf{k}", name=f"wf{k}_{j}", bufs=2)
                    nc.sync.dma_start(wf[k][:], dWX[j, k])

                dist = wpool.tile([L, B, D], f32, tag="dist")
                nc.scalar.activation(dist[:], tT[:], AF.Abs, bias=tRN[:, j : j + 1])
                ebf = wpool.tile([L, B, D], bf16, tag="ebf")
                nc.scalar.activation(ebf[:], dist[:], AF.Exp, scale=tAN[:, j : j + 1])

                q = wpool.tile([L, B, D], bf16, tag="q")
                a0 = wpool.tile([L, B, D], bf16, tag="a0")
                a2 = wpool.tile([L, B, D], bf16, tag="a2")
                a3 = wpool.tile([L, B, D], bf16, tag="a3")
                a4 = wpool.tile([L, B, D], bf16, tag="a4")
                for hb in range(2):
                    s = slice(hb * H, (hb + 1) * H)
                    nc.vector.tensor_mul(q[:, s, :], tXP[:, s, :], wf[1][:])
                    nc.vector.tensor_mul(a0[:, s, :], tX[:, s, :], wf[0][:])
                    nc.gpsimd.tensor_mul(a2[:, s, :], tM[:, s, :], wf[2][:])
                    nc.gpsimd.tensor_mul(a3[:, s, :], tDT[:, s, :], wf[3][:])
                    nc.vector.tensor_mul(a4[:, s, :], tP[:, s, :], wf[4][:])

                t = wpool.tile([L, B, D], bf16, tag="t")
                nc.vector.tensor_mul(t[:], q[:], ebf[:])
                s1 = wpool.tile([L, B, D], bf16, tag="s1")
                nc.vector.tensor_add(s1[:], a0[:], a4[:])
                s3 = wpool.tile([L, B, D], bf16, tag="s3")
                nc.vector.tensor_add(s3[:], s1[:], a2[:])
                s5 = wpool.tile([L, B, D], bf16, tag="s5")
                nc.vector.tensor_add(s5[:], s3[:], a3[:])
                s4 = wpool.tile([L, B, D], bf16, tag="s4")
                for hb in range(2):
                    s = slice(hb * H, (hb + 1) * H)
                    nc.vector.tensor_add(s4[:, s, :], s5[:, s, :], wf[5][:])
                p = wpool.tile([L, B, D], bf16, tag="p")
                nc.vector.tensor_add(p[:], s4[:], t[:])

                h = wpool.tile([L, B, D], bf16, tag="h")
                nc.scalar.activation(h[:], p[:], AF.Relu)
                wh = wpool.tile([L, B, D], bf16, tag="wh")
                for hb in range(2):
                    s = slice(hb * H, (hb + 1) * H)
                    nc.vector.tensor_mul(wh[:, s, :], h[:, s, :], wf[6][:])

                for c in range(NCH):
                    nc.tensor.matmul(
                        psc[c][:],
                        tOH[:, j, :],
                        wh[:, c * BC : (c + 1) * BC, :],
                        start=False,
                        stop=(j == RL - 1),
                    )

            # epilogue: relu(psum) -> sbuf f32, DMA out
            outf = opool.tile([RL, B, D], mybir.dt.float32, tag="outf")
            for c in range(NCH):
                nc.scalar.activation(
                    outf[:, c * BC : (c + 1) * BC, :], psc[c][:], AF.Relu
                )
            nc.sync.dma_start(dOUT.transpose([1, 0, 2]), outf[:])

    nc.compile()
    return nc


def _prep(X, T, M, DT, P, alpha, w_t, b_t, w_v, b_v):
    """Host-side shard prep: returns in_maps for the 8 cores."""
    refs = np.linspace(INIT_TIME, MAX_TS, R, dtype=np.float32)
    arelu = np.maximum(alpha.reshape(R).astype(np.float32), 0.0)

    Tt = np.ascontiguousarray(T.transpose(1, 0, 2)).astype(np.float32)
    Xb = np.ascontiguousarray(X.transpose(1, 0, 2)).astype(BF16)
    Mb = np.ascontiguousarray(M.transpose(1, 0, 2)).astype(BF16)
    DTb = np.ascontiguousarray(DT.transpose(1, 0, 2)).astype(BF16)
    Pb = np.ascontiguousarray(P.transpose(1, 0, 2)).astype(BF16)
    id48 = np.eye(D, dtype=np.float32).astype(BF16)

    # WX[j, k, l, b16, d]: weights expanded over 16 b (b-independent, reused
    # for both halves). k: 0..4 = w_t channels (ch1 slot holds w1), 5 = 5*b_t, 6 = w_v.
    wk_full = np.concatenate([w_t, 5.0 * b_t, w_v[..., None]], axis=3)  # [R, L, D, 7]
    in_maps = []
    for i in range(8):
        r0 = i * RL
        wx = wk_full[r0 : r0 + RL].transpose(0, 3, 1, 2)  # [RL, 7, L, D]
        wx = np.ascontiguousarray(
            np.broadcast_to(wx[:, :, :, None, :], (RL, 7, L, 16, D))
        ).astype(BF16)
        rn = np.broadcast_to(-refs[r0 : r0 + RL], (L, RL)).astype(np.float32)
        an = np.broadcast_to(-arelu[r0 : r0 + RL], (L, RL)).astype(np.float32)
        bvl = np.ascontiguousarray(
            (128.0 * b_v[r0 : r0 + RL, 0, :]).T
        ).astype(BF16)  # [D, RL]
        in_maps.append(
            {
                "Tt": Tt,
                "Xb": Xb,
                "Mb": Mb,
                "DTb": DTb,
                "Pb": Pb,
                "WX": wx,
                "RN": np.ascontiguousarray(rn),
                "AN": np.ascontiguousarray(an),
                "BVl": bvl,
                "ID48": id48,
            }
        )
    return in_maps


def run(trace=False, **inputs):
    if "nc" not in _CACHE:
        _CACHE["nc"] = _build()
    nc = _CACHE["nc"]
    in_maps = _prep(**inputs)
    res = run_bass_kernel_spmd(nc, in_maps, core_ids=list(range(8)), trace=trace)
    out = np.empty((B, R, D), dtype=np.float32)
    for i in range(8):
        out[:, i * RL : (i + 1) * RL, :] = res.results[i]["out"]
    return out, res


def kernel(**inputs) -> np.ndarray:
    out, _ = run(trace=False, **inputs)
    return out


# revision 5
# speedup vs baseline: 1.4232x; 1.1003x over previous
"""ALNN layer on 8 TRN2 NeuronCores.

Math (per reference):
  ref_r = linspace(0, 48, 64);  a_r = relu(alpha_r)
  e[b,r,l,d]  = exp(-a_r * |T[b,l,d] - ref_r|)
  intensity   = relu(X * e) = relu(X) * e            (e > 0 always)
  p[b,r,l,d]  = w0*X + w1*relu(X)*e + w2*M + w3*DT + w4*P + 5*b_t[r,l,d]
  h           = relu(p)
  out[b,r,d]  = relu( sum_l w_v[r,l,d]*h + 128*b_v[r,d] )

Sharding: R=64 split 8 ways (8 r per core); inputs replicated.
Layout: partition = L (128), free = (b, d) = (32, 48); loop over the 8 local r.
Engines: ACT does Abs/Exp/Relu (per-r scalars as [P,1] bias/scale);
DVE + GPSIMD split the 12 tensor-tensor ops; PE does the sum over L via
one-hot-column matmuls accumulating all 8 r into one PSUM tile, with the
128*b_v bias folded in as an identity-matmul that opens the accumulation.
"""
import sys

import numpy as np

if "/opt/trn_rl_repo" not in sys.path:
    sys.path.insert(0, "/opt/trn_rl_repo")

import ml_dtypes

from concourse import bacc, mybir
import concourse.tile as tile
from concourse.bass_utils import run_bass_kernel_spmd

BF16 = ml_dtypes.bfloat16
B, L, D = 32, 128, 48
R = 64
RL = R // 8  # r per core
INIT_TIME, MAX_TS = 0.0, 48.0

_CACHE = {}


def _build():
    nc = bacc.Bacc("TRN2", target_bir_lowering=False, debug=False, num_devices=8)
    f32, bf16 = mybir.dt.float32, mybir.dt.bfloat16
    AF = mybir.ActivationFunctionType

    # DRAM parameters (per-core shards / replicas)
    dTt = nc.dram_tensor("Tt", [L, B, D], f32, kind="ExternalInput").ap()
    dX = nc.dram_tensor("Xb", [L, B, D], bf16, kind="ExternalInput").ap()
    dM = nc.dram_tensor("Mb", [L, B, D], bf16, kind="ExternalInput").ap()
    dDT = nc.dram_tensor("DTb", [L, B, D], bf16, kind="ExternalInput").ap()
    dP = nc.dram_tensor("Pb", [L, B, D], bf16, kind="ExternalInput").ap()
    dWX = nc.dram_tensor("WX", [RL, 7, L, 16, D], bf16, kind="ExternalInput").ap()
    dRN = nc.dram_tensor("RN", [L, RL], f32, kind="ExternalInput").ap()
    dAN = nc.dram_tensor("AN", [L, RL], f32, kind="ExternalInput").ap()
    dBV = nc.dram_tensor("BVl", [D, RL], bf16, kind="ExternalInput").ap()
    dID = nc.dram_tensor("ID48", [D, D], bf16, kind="ExternalInput").ap()
    dOUT = nc.dram_tensor("out", [B, RL, D], f32, kind="ExternalOutput").ap()

    NCH = 4  # psum chunks of 8 b each (8*48 = 384 floats < 512/bank)
    BC = B // NCH

    with tile.TileContext(nc) as tc:
        with (
            tc.tile_pool(name="const", bufs=1) as cpool,
            tc.tile_pool(name="work", bufs=2) as wpool,
            tc.tile_pool(name="psum", bufs=1, space="PSUM") as ppool,
            tc.tile_pool(name="outp", bufs=1) as opool,
        ):
            # ---- load persistent inputs
            tT = cpool.tile([L, B, D], f32, tag="T")
            nc.sync.dma_start(tT[:], dTt)
            tX = cpool.tile([L, B, D], bf16, tag="X")
            nc.sync.dma_start(tX[:], dX)
            tM = cpool.tile([L, B, D], bf16, tag="M")
            nc.sync.dma_start(tM[:], dM)
            tDT = cpool.tile([L, B, D], bf16, tag="DT")
            nc.sync.dma_start(tDT[:], dDT)
            tP = cpool.tile([L, B, D], bf16, tag="P")
            nc.sync.dma_start(tP[:], dP)
            tRN = cpool.tile([L, RL], f32, tag="RN")
            nc.sync.dma_start(tRN[:], dRN)
            tAN = cpool.tile([L, RL], f32, tag="AN")
            nc.sync.dma_start(tAN[:], dAN)
            tBV = cpool.tile([D, RL], bf16, tag="BV")
            nc.sync.dma_start(tBV[:], dBV)
            tID = cpool.tile([D, D], bf16, tag="ID")
            nc.sync.dma_start(tID[:], dID)

            # xp = relu(X)
            tXP = cpool.tile([L, B, D], bf16, tag="XP")
            nc.scalar.activation(tXP[:], tX[:], AF.Relu)

            # one-hot lhsT matrices: oh[l, j, m] = (m == j)
            tOH = cpool.tile([L, RL, RL], bf16, tag="OH")
            nc.vector.memset(tOH[:], 0.0)
            for j in range(RL):
                nc.vector.memset(tOH[:, j, j : j + 1], 1.0)

            # psum accumulators, one bank-sized chunk of (b, d) each
            psc = [
                ppool.tile([RL, BC, D], mybir.dt.float32, tag=f"ps{c}", name=f"ps{c}")
                for c in range(NCH)
            ]
            # open each accumulation group with the bias term:
            # ps[r, b, d] = 128*b_v[r, d] via lhsT=BVl[d', r], rhs=Id[d', (b d)]
            for c in range(NCH):
                nc.tensor.matmul(
                    psc[c][:],
                    tBV[:],
                    tID[:, None, :].to_broadcast((D, BC, D)),
                    start=True,
                    stop=False,
                )

            H = 16  # weights expanded over half of B; ops run per b-half
            for j in range(RL):
                # per-r weights, host-pre-expanded to flat [L, 16, D] bf16 tiles
                wf = {}
                for k in range(7):
                    wf[k] = wpool.tile([L, H, D], bf16, tag=f"wf{k}", name=f"wf{k}_{j}", bufs=2)
                    nc.sync.dma_start(wf[k][:], dWX[j, k])

                dist = wpool.tile([L, B, D], f32, tag="dist")
                nc.scalar.activation(dist[:], tT[:], AF.Abs, bias=tRN[:, j : j + 1])
                ebf = wpool.tile([L, B, D], bf16, tag="ebf")
                nc.scalar.activation(ebf[:], dist[:], AF.Exp, scale=tAN[:, j : j + 1])

                q = wpool.tile([L, B, D], bf16, tag="q")
                a0 = wpool.tile([L, B, D], bf16, tag="a0")
                a2 = wpool.tile([L, B, D], bf16, tag="a2")
                a3 = wpool.tile([L, B, D], bf16, tag="a3")
                a4 = wpool.tile([L, B, D], bf16, tag="a4")
                for hb in range(2):
                    s = slice(hb * H, (hb + 1) * H)
                    nc.vector.tensor_mul(q[:, s, :], tXP[:, s, :], wf[1][:])
                    nc.vector.tensor_mul(a0[:, s, :], tX[:, s, :], wf[0][:])
                    nc.vector.tensor_mul(a2[:, s, :], tM[:, s, :], wf[2][:])
                    nc.vector.tensor_mul(a3[:, s, :], tDT[:, s, :], wf[3][:])
                    nc.vector.tensor_mul(a4[:, s, :], tP[:, s, :], wf[4][:])

                t = wpool.tile([L, B, D], bf16, tag="t")
                nc.vector.tensor_mul(t[:], q[:], ebf[:])
                s1 = wpool.tile([L, B, D], bf16, tag="s1")
                nc.vector.tensor_add(s1[:], a0[:], a4[:])
                s3 = wpool.tile([L, B, D], bf16, tag="s3")
                nc.vector.tensor_add(s3[:], s1[:], a2[:])
                s5 = wpool.tile([L, B, D], bf16, tag="s5")
                nc.vector.tensor_add(s5[:], s3[:], a3[:])
                s4 = wpool.tile([L, B, D], bf16, tag="s4")
                for hb in range(2):
                    s = slice(hb * H, (hb + 1) * H)
                    nc.vector.tensor_add(s4[:, s, :], s5[:, s, :], wf[5][:])
                p = wpool.tile([L, B, D], bf16, tag="p")
                nc.vector.tensor_add(p[:], s4[:], t[:])

                h = wpool.tile([L, B, D], bf16, tag="h")
                nc.scalar.activation(h[:], p[:], AF.Relu)
                wh = wpool.tile([L, B, D], bf16, tag="wh")
                for hb in range(2):
                    s = slice(hb * H, (hb + 1) * H)
                    nc.vector.tensor_mul(wh[:, s, :], h[:, s, :], wf[6][:])

                for c in range(NCH):
                    nc.tensor.matmul(
                        psc[c][:],
                        tOH[:, j, :],
                        wh[:, c * BC : (c + 1) * BC, :],
                        start=False,
                        stop=(j == RL - 1),
                    )

            # epilogue: relu(psum) -> sbuf f32, DMA out
            outf = opool.tile([RL, B, D], mybir.dt.float32, tag="outf")
            for c in range(NCH):
                nc.scalar.activation(
                    outf[:, c * BC : (c + 1) * BC, :], psc[c][:], AF.Relu
                )
            nc.sync.dma_start(dOUT.transpose([1, 0, 2]), outf[:])

    nc.compile()
    return nc


def _prep(X, T, M, DT, P, alpha, w_t, b_t, w_v, b_v):
    """Host-side shard prep: returns in_maps for the 8 cores."""
    refs = np.linspace(INIT_TIME, MAX_TS, R, dtype=np.float32)
    arelu = np.maximum(alpha.reshape(R).astype(np.float32), 0.0)

    Tt = np.ascontiguousarray(T.transpose(1, 0, 2)).astype(np.float32)
    Xb = np.ascontiguousarray(X.transpose(1, 0, 2)).astype(BF16)
    Mb = np.ascontiguousarray(M.transpose(1, 0, 2)).astype(BF16)
    DTb = np.ascontiguousarray(DT.transpose(1, 0, 2)).astype(BF16)
    Pb = np.ascontiguousarray(P.transpose(1, 0, 2)).astype(BF16)
    id48 = np.eye(D, dtype=np.float32).astype(BF16)

    # WX[j, k, l, b16, d]: weights expanded over 16 b (b-independent, reused
    # for both halves). k: 0..4 = w_t channels (ch1 slot holds w1), 5 = 5*b_t, 6 = w_v.
    wk_full = np.concatenate([w_t, 5.0 * b_t, w_v[..., None]], axis=3)  # [R, L, D, 7]
    in_maps = []
    for i in range(8):
        r0 = i * RL
        wx = wk_full[r0 : r0 + RL].transpose(0, 3, 1, 2)  # [RL, 7, L, D]
        wx = np.ascontiguousarray(
            np.broadcast_to(wx[:, :, :, None, :], (RL, 7, L, 16, D))
        ).astype(BF16)
        rn = np.broadcast_to(-refs[r0 : r0 + RL], (L, RL)).astype(np.float32)
        an = np.broadcast_to(-arelu[r0 : r0 + RL], (L, RL)).astype(np.float32)
        bvl = np.ascontiguousarray(
            (128.0 * b_v[r0 : r0 + RL, 0, :]).T
        ).astype(BF16)  # [D, RL]
        in_maps.append(
            {
                "Tt": Tt,
                "Xb": Xb,
                "Mb": Mb,
                "DTb": DTb,
                "Pb": Pb,
                "WX": wx,
                "RN": np.ascontiguousarray(rn),
                "AN": np.ascontiguousarray(an),
                "BVl": bvl,
                "ID48": id48,
            }
        )
    return in_maps


def run(trace=False, **inputs):
    if "nc" not in _CACHE:
        _CACHE["nc"] = _build()
    nc = _CACHE["nc"]
    in_maps = _prep(**inputs)
    res = run_bass_kernel_spmd(nc, in_maps, core_ids=list(range(8)), trace=trace)
    out = np.empty((B, R, D), dtype=np.float32)
    for i in range(8):
        out[:, i * RL : (i + 1) * RL, :] = res.results[i]["out"]
    return out, res


def kernel(**inputs) -> np.ndarray:
    out, _ = run(trace=False, **inputs)
    return out
